# revision 5
# baseline (speedup 1.0000x reference)
"""Trainium2 Bass kernel for ContextQueryAttention (BiDAF-style trilinear
attention). Data-parallel over batch across 8 NeuronCores (4 batches/core).

Per batch (c=1024 context rows, q=128 query rows, h=256 hidden):
  S[c,q]   = ctx@cw + (qry@qw)^T + (ctx*cqw)@qry^T + bias
  S_bar    = softmax_c(S); S_bar_bar = softmax_q(S)
  A        = S @ qry
  B        = S_bar @ (S_bar_bar^T @ ctx)
  out      = concat([ctx, A, ctx*A, ctx*B], -1)

v7: engine-load cuts + fp8 ctxB output + PE warmup + streaming stores.
  - s1+bias applied as per-partition activation bias on the exp AND as a
    scalar add on the raw-S evac (kills the rank-1 rider matmuls).
  - zc (softmax_q denominators) via 8 tiny ones-column matmuls on the PE
    (kills the DVE reduce); e_sb = tr * rc via broadcast tensor_tensor
    (stride-0 in1), one op per 4-tile transpose wave.
  - ctx*B stored as fp8 (values are convex-combos of ctx, |x| << 240).
  - A, ctxA, ctxB in three separate contiguous HBM tensors; A stored
    right after its ACT evac (before the Pool multiply), ctxA after
    Pool, ctxB after the deferred B phase. All stores stream per batch.
  - PE warmup spin on memset garbage (no DMA dependency) so HAM
    un-throttles (1.2 -> 2.4 GHz) before the first real matmul.
  - batch-0 loads on the ACT HWDGE ring; everything else on sync.
"""

import numpy as np

B, C, Q, H = 32, 1024, 128, 256
N_CORES = 8
BPC = B // N_CORES  # batches per core
P = 128
HC = H // P  # h chunks of 128
CT = C // P  # c tiles of 128
CCH = 512  # S^T free-dim chunk (1 PSUM bank of fp32)
NCC = C // CCH

_NC_CACHE = {}


def _build_kernel():
    import concourse.bacc as bacc
    import concourse.tile as tile
    from concourse import mybir
    from concourse.bass import broadcast_tensor_aps

    f32 = mybir.dt.float32
    bf16 = mybir.dt.bfloat16
    fp8 = mybir.dt.float8e4
    AF = mybir.ActivationFunctionType
    ALU = mybir.AluOpType

    nc = bacc.Bacc(trn_type="TRN2", target_bir_lowering=False, debug=False)
    # consts packed: qtw = [qry^T cols | qw cols | identity | ones-col]
    QTW_W = HC * BPC * Q + HC + P + 1
    qtw_d = nc.dram_tensor("qtw", [P, QTW_W], bf16, kind="ExternalInput").ap()
    qa_d = nc.dram_tensor("qa", [P, BPC * H], bf16, kind="ExternalInput").ap()
    # packed f32 consts: [0:HC]=cqw cols, [HC:2HC]=cw cols, [2HC]=bias
    wv_d = nc.dram_tensor("wv", [P, 2 * HC + 1], f32, kind="ExternalInput").ap()
    # per-batch inputs
    ctx_d = nc.dram_tensor("ctx", [BPC, P, CT * H], bf16, kind="ExternalInput").ap()
    ctxT_d = nc.dram_tensor("ctxT", [BPC, P, HC * C], bf16, kind="ExternalInput").ap()
    # outputs: separate contiguous tensors per channel
    oA_d = nc.dram_tensor("oA", [BPC, P, CT * H], bf16, kind="ExternalOutput").ap()
    oCA_d = nc.dram_tensor("oCA", [BPC, P, CT * H], bf16, kind="ExternalOutput").ap()
    oB_d = nc.dram_tensor("oB", [BPC, P, CT * H], fp8, kind="ExternalOutput").ap()

    from contextlib import ExitStack

    with tile.TileContext(nc) as tc, ExitStack() as es:
        consts = es.enter_context(tc.tile_pool(name="consts", bufs=1))
        p_cxt = es.enter_context(tc.tile_pool(name="p_cxt", bufs=3))
        p_cxn = es.enter_context(tc.tile_pool(name="p_cxn", bufs=3))
        p_et = es.enter_context(tc.tile_pool(name="p_et", bufs=2))
        p_sr = es.enter_context(tc.tile_pool(name="p_sr", bufs=2))
        p_esb = es.enter_context(tc.tile_pool(name="p_esb", bufs=2))
        p_oA = es.enter_context(tc.tile_pool(name="p_oA", bufs=2))
        p_oCA = es.enter_context(tc.tile_pool(name="p_oCA", bufs=2))
        p_oB = es.enter_context(tc.tile_pool(name="p_oB", bufs=2))
        p_vec = es.enter_context(tc.tile_pool(name="p_vec", bufs=3))
        # PSUM banks: stp 2x[P,512]f32 (2) + tr 2x[P,4,128]bf16 (2)
        #             + tz 1x[P,264]f32 (1) + ab 3x[P,2,256]f32 (3)
        pp_st = es.enter_context(tc.tile_pool(name="pp_st", bufs=2, space="PSUM"))
        pp_tr = es.enter_context(tc.tile_pool(name="pp_tr", bufs=2, space="PSUM"))
        pp_tz = es.enter_context(tc.tile_pool(name="pp_tz", bufs=1, space="PSUM"))
        pp_ab = es.enter_context(tc.tile_pool(name="pp_ab", bufs=3, space="PSUM"))

        # ---- PE warmup spin on memset garbage (no DMA dependency): keeps
        # PE busy ~3.5us so HAM un-throttles before the first real matmul ----
        warm_src = consts.tile([P, P], bf16)
        nc.gpsimd.memset(warm_src, 1.0)
        for w in range(2):
            tr_warm = pp_tr.tile([P, 4, P], bf16, tag="tr8", name=f"warm{w}")
            for i in range(18):
                nc.tensor.matmul(
                    tr_warm[:, i % 4, :],
                    lhsT=warm_src,
                    rhs=warm_src,
                    is_transpose=True,
                )

        # ---- const DMAs (qtw first: feeds the preamble) ----
        qtw = consts.tile([P, QTW_W], bf16)
        nc.sync.dma_start(out=qtw, in_=qtw_d)
        wv = consts.tile([P, 2 * HC + 1], f32)
        nc.sync.dma_start(out=wv, in_=wv_d)
        qa_sb = consts.tile([P, BPC * H], bf16)
        nc.sync.dma_start(out=qa_sb, in_=qa_d)
        qt_all = qtw[:, 0 : HC * BPC * Q].rearrange("p (j bq) -> p j bq", j=HC)
        qa_all = qa_sb.rearrange("p (b h) -> p b h", b=BPC)
        qwr = qtw[:, HC * BPC * Q : HC * BPC * Q + HC]
        idones = qtw[:, HC * BPC * Q + HC :]  # [P, 129] = [I | 1]
        ident = idones[:, 0:P]
        ones_col = idones[:, P : P + 1]
        bias_col = wv[:, 2 * HC : 2 * HC + 1]

        # ---- preamble: s1 columns (s1[q] = qry@qw + bias) via tiny matmuls;
        # qt_cq = qry^T*cqw + cw on DVE ----
        s1p = pp_tz.tile([P, 264], f32, tag="tz", name="s1p")
        for b in range(BPC):
            for j in range(HC):
                nc.tensor.matmul(
                    s1p[:, 256 + b : 257 + b],
                    lhsT=qt_all[:, j, b * Q : (b + 1) * Q],
                    rhs=qwr[:, j : j + 1],
                    start=(j == 0),
                    stop=(j == HC - 1),
                )
        s1c = consts.tile([P, BPC], f32)
        nc.scalar.activation(
            s1c, s1p[:, 256 : 256 + BPC], AF.Identity, bias=bias_col, scale=1.0
        )

        qt_cq = consts.tile([P, HC, BPC * Q], bf16)
        for j in range(HC):
            nc.vector.tensor_scalar(
                qt_cq[:, j],
                qt_all[:, j],
                wv[:, j : j + 1],
                wv[:, HC + j : HC + j + 1],
                ALU.mult,
                ALU.add,
            )

        # ---- per-batch loads (batch 0 on ACT ring, rest on sync) ----
        def load_batch(b, q):
            ctxT_t = p_cxt.tile([P, HC, C], bf16, tag="cxt", name=f"cxt{b}")
            ctx_t = p_cxn.tile([P, CT, H], bf16, tag="cxn", name=f"cxn{b}")
            q.dma_start(out=ctxT_t.rearrange("p j c -> p (j c)"), in_=ctxT_d[b])
            q.dma_start(out=ctx_t.rearrange("p t h -> p (t h)"), in_=ctx_d[b])
            return ctxT_t, ctx_t

        tiles = {0: load_batch(0, nc.scalar), 1: load_batch(1, nc.sync)}

        # cross-iteration state of batch b-1: (b, e_t, e_sb, rq, ctx_t, oB_t)
        prev = None

        def emit_t_phase(state, tz):
            """T(b-1) = S_bar_bar^T @ ctx into tz[:,0:H]; ts = T * rq."""
            bp, e_tp, e_sbp, rqp, ctx_tp, _ = state
            for t in range(CT):
                nc.tensor.matmul(
                    tz[:, 0:H],
                    lhsT=e_sbp[:, t // 4, t % 4, :],
                    rhs=ctx_tp[:, t, :],
                    start=(t == 0),
                    stop=(t == CT - 1),
                )
            ts = p_vec.tile([P, H], bf16, tag="ts", name=f"ts{bp}")
            nc.vector.tensor_scalar_mul(ts, tz[:, 0:H], rqp)
            return ts

        def emit_b_phase(state, ts):
            """B(b-1) pairs + ctx*B (fp8 out) on DVE, then store oB(b-1)
            in two halves so the first streams while the second computes."""
            bp, e_tp, _, _, ctx_tp, oB_tp = state
            for p2 in range(CT // 2):
                t0 = 2 * p2
                pb = pp_ab.tile([P, 2, H], f32, tag="ab", name=f"pb{bp}{p2}")
                for k in range(2):
                    nc.tensor.matmul(
                        pb[:, k, :],
                        lhsT=e_tp[:, (t0 + k) * P : (t0 + k + 1) * P],
                        rhs=ts,
                        start=True,
                        stop=True,
                    )
                nc.vector.tensor_mul(
                    oB_tp[:, t0 : t0 + 2, :], ctx_tp[:, t0 : t0 + 2, :], pb
                )
                if p2 == 1:
                    nc.sync.dma_start(
                        out=oB_d[bp, :, 0 : 4 * H],
                        in_=oB_tp[:, 0:4, :].rearrange("p t h -> p (t h)"),
                    )
            nc.sync.dma_start(
                out=oB_d[bp, :, 4 * H :],
                in_=oB_tp[:, 4:CT, :].rearrange("p t h -> p (t h)"),
            )

        for b in range(BPC):
            if b + 2 < BPC:
                tiles[b + 2] = load_batch(b + 2, nc.sync)
            ctxT_t, ctx_t = tiles[b]
            bq = slice(b * Q, (b + 1) * Q)
            s1_b = s1c[:, b : b + 1]

            # ---- S^T chunks; exp with s1 bias + fused row-sums; raw evac
            # (+s1) split ACT/DVE ----
            e_t = p_et.tile([P, C], bf16, tag="e_t")
            st_raw = p_sr.tile([P, C], bf16, tag="st_raw")
            rsum = p_vec.tile([P, NCC], f32, tag="rsum")
            for cc in range(NCC):
                sl = slice(cc * CCH, (cc + 1) * CCH)
                stp = pp_st.tile([P, CCH], f32, tag="stp")
                for j in range(HC):
                    nc.tensor.matmul(
                        stp,
                        lhsT=qt_cq[:, j, bq],
                        rhs=ctxT_t[:, j, sl],
                        start=(j == 0),
                        stop=(j == HC - 1),
                    )
                nc.scalar.activation(
                    e_t[:, sl],
                    stp,
                    AF.Exp,
                    bias=s1_b,
                    accum_out=rsum[:, cc : cc + 1],
                )
                if cc == 0:
                    nc.vector.tensor_scalar_add(st_raw[:, sl], stp, s1_b)
                else:
                    nc.scalar.activation(
                        st_raw[:, sl], stp, AF.Identity, bias=s1_b, scale=1.0
                    )

            # softmax_c denominators: zq = rsum0+rsum1 (Pool), rq = 1/zq (DVE)
            zq = p_vec.tile([P, 1], f32, tag="zq")
            nc.gpsimd.tensor_add(zq, rsum[:, 0:1], rsum[:, 1:2])
            rq = p_vec.tile([P, 1], f32, tag="rq")
            nc.vector.reciprocal(rq, zq)

            # shared tz bank this iteration: T(b-1) in [:,0:H], zc(b) in
            # [:,H:H+CT]
            tz = pp_tz.tile([P, 264], f32, tag="tz", name=f"tz{b}")

            # ---- deferred T-phase of batch b-1 ----
            ts_prev = emit_t_phase(prev, tz) if prev is not None else None

            oA_t = p_oA.tile([P, CT, H], bf16, tag="oA")
            oCA_t = p_oCA.tile([P, CT, H], bf16, tag="oCA")

            def emit_a_phase():
                # A = S_raw @ qry per c-tile pair; evac on ACT, ctx*A on Pool
                for p2 in range(CT // 2):
                    t0 = 2 * p2
                    pa = pp_ab.tile([P, 2, H], f32, tag="ab", name=f"pa{b}{p2}")
                    for k in range(2):
                        nc.tensor.matmul(
                            pa[:, k, :],
                            lhsT=st_raw[:, (t0 + k) * P : (t0 + k + 1) * P],
                            rhs=qa_all[:, b, :],
                            start=True,
                            stop=True,
                        )
                    nc.scalar.copy(oA_t[:, t0 : t0 + 2, :], pa)
                    ctxa_engine = nc.vector if b == BPC - 1 else nc.gpsimd
                    ctxa_engine.tensor_mul(
                        oCA_t[:, t0 : t0 + 2, :],
                        ctx_t[:, t0 : t0 + 2, :],
                        oA_t[:, t0 : t0 + 2, :],
                    )
                # A channel is complete after the ACT evacs; store it before
                # the slower Pool products finish
                nc.sync.dma_start(out=oA_d[b], in_=oA_t.rearrange("p t h -> p (t h)"))

            def emit_tr_es():
                # transposes of e_t (two 4-tile waves) + zc ones-matmuls on
                # PE; e_sb = tr * (1/zc) via broadcast TT, one op per wave
                e_sb = p_esb.tile([P, NCC, 4, P], bf16, tag="e_sb")
                rc8 = p_vec.tile([P, NCC, 4], f32, tag="rc8")
                for w in range(NCC):
                    tr4 = pp_tr.tile([P, 4, P], bf16, tag="tr8", name=f"tr{b}{w}")
                    for t in range(4):
                        tt = 4 * w + t
                        nc.tensor.matmul(
                            tr4[:, t, :],
                            lhsT=e_t[:, tt * P : (tt + 1) * P],
                            rhs=ident,
                            is_transpose=True,
                        )
                        nc.tensor.matmul(
                            tz[:, H + tt : H + tt + 1],
                            lhsT=e_t[:, tt * P : (tt + 1) * P],
                            rhs=ones_col,
                            start=True,
                            stop=True,
                        )
                    nc.vector.reciprocal(rc8[:, w], tz[:, H + 4 * w : H + 4 * w + 4])
                    in0, in1 = broadcast_tensor_aps(tr4, rc8[:, w, :, None])
                    nc.vector.tensor_tensor(e_sb[:, w], in0, in1, ALU.mult)
                return e_sb

            if b == BPC - 1:
                # last batch: es chain first so the epilogue T starts early
                e_sb = emit_tr_es()
                emit_a_phase()
            else:
                emit_a_phase()
                e_sb = emit_tr_es()

            # ---- deferred B-phase + ctxB store of batch b-1 ----
            if prev is not None:
                emit_b_phase(prev, ts_prev)

            # ---- store ctxA of batch b (ready after the Pool products) ----
            nc.sync.dma_start(out=oCA_d[b], in_=oCA_t.rearrange("p t h -> p (t h)"))

            oB_t = p_oB.tile([P, CT, H], fp8, tag="oB")
            prev = (b, e_t, e_sb, rq, ctx_t, oB_t)

        # ---- epilogue: T/B/ctxB/store for the last batch ----
        tz = pp_tz.tile([P, 264], f32, tag="tz", name="tzL")
        ts_last = emit_t_phase(prev, tz)
        emit_b_phase(prev, ts_last)

    nc.compile()
    return nc


def _get_nc():
    if "nc" not in _NC_CACHE:
        _NC_CACHE["nc"] = _build_kernel()
    return _NC_CACHE["nc"]


def make_in_maps(context, query, c_weight, q_weight, cq_weight, bias):
    import ml_dtypes

    bf16 = ml_dtypes.bfloat16
    context = np.ascontiguousarray(np.asarray(context, dtype=np.float32))
    query = np.asarray(query, dtype=np.float32)
    cw = np.asarray(c_weight, dtype=np.float32).reshape(H)
    qw = np.asarray(q_weight, dtype=np.float32).reshape(H)
    cqw = np.asarray(cq_weight, dtype=np.float32).reshape(H)
    bs = float(np.asarray(bias, dtype=np.float32).reshape(1)[0])

    # wv: [:, 0:HC]=cqw cols, [:, HC:2HC]=cw cols, [:, 2HC]=bias (col j is h=j*128+p)
    wv = np.ascontiguousarray(
        np.concatenate(
            [
                cqw.reshape(HC, P).T,
                cw.reshape(HC, P).T,
                np.full((P, 1), bs, np.float32),
            ],
            axis=1,
        ).astype(np.float32)
    )
    qwr = qw.reshape(HC, P).T.astype(bf16)
    idones = np.concatenate(
        [np.eye(P, dtype=np.float32), np.ones((P, 1), np.float32)], axis=1
    ).astype(bf16)

    in_maps = []
    for i in range(N_CORES):
        sl = slice(i * BPC, (i + 1) * BPC)
        ctx_i = context[sl]
        qry_i = query[sl]
        # ctx: [b, c, h] -> [b, p, t, h] with c = t*128+p
        ctx_s = np.ascontiguousarray(
            ctx_i.reshape(BPC, CT, P, H).transpose(0, 2, 1, 3).reshape(BPC, P, CT * H)
        ).astype(bf16)
        # ctxT: [b, h, c] -> [b, p, j, c] with h = j*128+p
        ctxT_s = np.ascontiguousarray(
            ctx_i.transpose(0, 2, 1)
            .reshape(BPC, HC, P, C)
            .transpose(0, 2, 1, 3)
            .reshape(BPC, P, HC * C)
        ).astype(bf16)
        # qry^T: [b, h, q] -> [p, j, b, q]
        qt_s = (
            qry_i.transpose(0, 2, 1)
            .reshape(BPC, HC, P, Q)
            .transpose(2, 1, 0, 3)
            .reshape(P, HC * BPC * Q)
        ).astype(bf16)
        # qry: [b, q, h] -> [q, b, h]
        qa_s = qry_i.transpose(1, 0, 2).reshape(P, BPC * H).astype(bf16)
        qtw = np.ascontiguousarray(np.concatenate([qt_s, qwr, idones], axis=1))
        qa_c = np.ascontiguousarray(qa_s)
        in_maps.append(
            {"ctx": ctx_s, "ctxT": ctxT_s, "qtw": qtw, "qa": qa_c, "wv": wv}
        )
    return in_maps


def kernel(context, query, c_mask, q_mask, c_weight, q_weight, cq_weight, bias):
    from concourse import bass_utils

    nc = _get_nc()
    in_maps = make_in_maps(context, query, c_weight, q_weight, cq_weight, bias)
    res = bass_utils.run_bass_kernel_spmd(nc, in_maps, core_ids=list(range(N_CORES)))

    context = np.asarray(context, dtype=np.float32)
    full = np.empty((B, C, 4 * H), dtype=np.float32)
    full[:, :, 0:H] = context

    def unshard(name, i):
        return (
            res.results[i][name]
            .reshape(BPC, P, CT, H)
            .transpose(0, 2, 1, 3)
            .reshape(BPC, C, H)
            .astype(np.float32)
        )

    for i in range(N_CORES):
        sl = slice(i * BPC, (i + 1) * BPC)
        full[sl, :, H : 2 * H] = unshard("oA", i)
        full[sl, :, 2 * H : 3 * H] = unshard("oCA", i)
        full[sl, :, 3 * H :] = unshard("oB", i)
    return full
